# revision 9
# baseline (speedup 1.0000x reference)
"""Trainium2 kernel for BinaryXnorExceptOutliersLinear.

Computes  out = x @ w_sim.T + bias  where
  w_sim = where(outlier_mask, weight, sign(weight) * binary_scale)

Distribution: column-parallel over 8 NeuronCores — weight / outlier_mask /
bias are sharded along out_features (11008 -> 8 x 1376), x is replicated,
each core produces its [8192, 1376] output slice, concatenated on host.

Per-core kernel (v3):
  1. x transposes run on the DMA XBAR from a DRAM bf16 staging copy
     (gpsimd cast-DMA f32->bf16 DRAM->DRAM, then one dma_start_transpose
     per (512-token block, k-tile) — the XBAR queue cost is ~1.24us per
     instruction regardless of size, so big blocks amortize it).
  2. Weight prep (once): DMA weight+mask shard, sign+scale on ACT, outlier
     restore via DVE copy_predicated, PE transpose (identity matmul) into
     the SBUF-resident [K, 1376] bf16 wT.  PE transposes here keep the
     staged matmuls' wT dependency on ordinary engine semaphores.
  3. Main loop: 3x32 accumulating matmuls per token tile (chunks
     512/512/352 over out-features, 32 k-tiles) emitted kt-major so the
     three chunk matmuls of one (tile, k-tile) share the PE stationary;
     DVE adds bias on the PSUM->SBUF copy; gpsimd DMA out.
  4. A post-build pass deletes InstLdweights instructions whose stationary
     operand is already resident in the PE array, cutting PE weight-load
     overhead by ~2/3.
"""

import json
import sys

for _p in ("/opt/trn_rl_repo",):
    if _p not in sys.path:
        sys.path.insert(0, _p)

import ml_dtypes
import numpy as np

import concourse.bass as bass
import concourse.mybir as mybir
from concourse.tile import TileContext
from concourse.bass_utils import run_bass_kernel_spmd

B, S, DIN, DOUT = 4, 2048, 4096, 11008
M = B * S              # 8192 tokens
NCORES = 8
DSH = DOUT // NCORES   # 1376 out-features per core
K = DIN
KT = K // 128          # 32 k-tiles
CHUNKS = [(0, 512), (512, 512), (1024, 352)]   # out-feature chunks per core
BLK = 512              # tokens per x-transpose block
BT = BLK // 128        # 4 token tiles per block

F32 = mybir.dt.float32
BF16 = mybir.dt.bfloat16
U8 = mybir.dt.uint8

MAX_WAITS = 1  # stock walrus: one sem-wait command per instruction


def _split_excess_waits(nc, max_waits: int = MAX_WAITS) -> int:
    """Stock AWS walrus rejects instructions with more than one sem-wait
    ("Too many sync wait commands"). Peel excess waits onto bare
    EventSemaphore stubs placed right before the instruction on the same
    engine (engines run their stream in order, so ordering is preserved)."""
    n_split = 0
    for f in nc.m.functions:
        for blk in f.blocks:
            il = blk.instructions
            out = []
            changed = False
            for inst in il:
                si = inst.sync_info
                waits = list(si.on_wait) if (si and si.on_wait) else []
                if len(waits) > max_waits:
                    changed = True
                    extra, keep = waits[:-max_waits], waits[-max_waits:]
                    for ci, start in enumerate(range(0, len(extra), max_waits)):
                        chunk = extra[start:start + max_waits]
                        stub = mybir.InstEventSemaphore(
                            name=f"{inst.name}_wsplit{ci}", ins=[], outs=[])
                        stub.engine = inst.engine
                        stub.sync_info = mybir.SyncInfo(
                            on_wait=list(chunk), on_update=[])
                        out.append(stub)
                        n_split += 1
                    si.on_wait = keep
                    inst.sync_info = si
                out.append(inst)
            if changed:
                il.clear()
                il.extend(out)
    return n_split


def _ldw_key(inst):
    """Stable key for an InstLdweights' weights operand (the stationary AP)."""
    try:
        j = json.loads(mybir.instruction_to_pretty_json_string(inst))
        return json.dumps(j.get("ins"), sort_keys=True)
    except Exception:
        return None


def _dedupe_ldweights(nc) -> int:
    """Delete InstLdweights whose weights AP is identical to the stationary
    already resident in the PE array (loaded by the previous InstLdweights on
    the PE stream, with only non-loading matmuls / events in between).
    Sync waits/updates of a deleted load are merged into the next PE
    instruction (its matmul)."""
    n_del = 0
    for f in nc.m.functions:
        for blk in f.blocks:
            il = list(blk.instructions)
            last_key = None
            del_idx = set()
            for i, inst in enumerate(il):
                if getattr(inst, "engine", None) != mybir.EngineType.PE:
                    continue
                if isinstance(inst, mybir.InstLdweights):
                    key = _ldw_key(inst)
                    if key is not None and key == last_key:
                        del_idx.add(i)
                        n_del += 1
                    else:
                        last_key = key
                elif isinstance(inst, mybir.InstMatmult):
                    if inst.ldweights is not False:
                        last_key = None  # self-loading matmul clobbers array
                elif isinstance(inst, (mybir.InstEventSemaphore,
                                       mybir.InstDrain)):
                    pass
                else:
                    last_key = None  # unknown PE instruction: be safe
            if not del_idx:
                continue
            out = []
            pend_w, pend_u = [], []
            for i, inst in enumerate(il):
                if i in del_idx:
                    si = inst.sync_info
                    if si and si.on_wait:
                        pend_w.extend(si.on_wait)
                    if si and si.on_update:
                        pend_u.extend(si.on_update)
                    continue
                if (pend_w or pend_u) and \
                        getattr(inst, "engine", None) == mybir.EngineType.PE:
                    si = inst.sync_info
                    w = list(si.on_wait) if (si and si.on_wait) else []
                    u = list(si.on_update) if (si and si.on_update) else []
                    inst.sync_info = mybir.SyncInfo(
                        on_wait=pend_w + w, on_update=u + pend_u)
                    pend_w, pend_u = [], []
                out.append(inst)
            assert not pend_w and not pend_u, "dangling waits from deleted LDW"
            blk.instructions.clear()
            blk.instructions.extend(out)
    return n_del


def build_nc(m_tokens: int = M):
    """Build the per-core Bass program (SPMD: same program on all cores)."""
    tok_tiles = m_tokens // 128
    n_blocks = m_tokens // BLK
    nc = bass.Bass()
    x_h = nc.declare_dram_parameter("x", [m_tokens, K], F32, isOutput=False)
    w_h = nc.declare_dram_parameter("weight", [DSH, K], F32, isOutput=False)
    b_h = nc.declare_dram_parameter("bias", [DSH], F32, isOutput=False)
    mk_h = nc.declare_dram_parameter("outlier_mask", [DSH, K], U8, isOutput=False)
    sc_h = nc.declare_dram_parameter("binary_scale", [1, 1], F32, isOutput=False)
    id_h = nc.declare_dram_parameter("identity_const", [128, 128], BF16,
                                     isOutput=False)
    out_h = nc.declare_dram_parameter("out", [m_tokens, DSH], F32, isOutput=True)
    xbf_h = nc.dram_tensor("x_bf", [m_tokens, K], BF16)

    dout_tiles = [(o, min(128, DSH - o)) for o in range(0, DSH, 128)]
    n_staged = min(BT, tok_tiles)

    with TileContext(nc) as tc:
        with tc.tile_pool(name="const", bufs=1) as const_pool:

            identity = const_pool.tile([128, 128], BF16)
            nc.scalar.dma_start(identity, id_h[:, :])
            scale_vec = const_pool.tile([128, 1], F32)
            nc.gpsimd.dma_start(out=scale_vec,
                                in_=sc_h[:, :].to_broadcast((128, 1)))
            bias_rep = const_pool.tile([128, DSH], F32)

            # Resident binarized+transposed weight: [k-in-tile, kt, dout]
            wT = const_pool.tile([128, KT * DSH], BF16)
            wT_r = wT.rearrange("p (kt d) -> p kt d", kt=KT)

            with tc.tile_pool(name="xtp", bufs=2) as xtp, \
                 tc.tile_pool(name="osbp", bufs=4) as osbp, \
                 tc.tile_pool(name="wprep", bufs=2) as wp, \
                 tc.tile_pool(name="mpsum", bufs=2, space="PSUM") as psum_pool:

                xtbs = {}
                osbs = {}
                psos_map = {}
                emitted = set()   # (t, ci) chunks whose matmuls are emitted

                def emit_cast(bk):
                    """Cast x block bk f32->bf16 into the DRAM staging copy."""
                    for q in range(2):
                        rows = slice(bk * BLK + q * (BLK // 2),
                                     bk * BLK + (q + 1) * (BLK // 2))
                        nc.gpsimd.dma_start(xbf_h[rows, :], x_h[rows, :])

                def emit_xbar(bk):
                    """XBAR-transpose block bk: [BLK, K] -> [K, BLK] tiles."""
                    xtb = xtp.tile([128, KT * BLK], BF16, tag="xtb",
                                   name="xtb")
                    rows = slice(bk * BLK, (bk + 1) * BLK)
                    for kt in range(KT):
                        nc.sync.dma_start_transpose(
                            xtb[:, kt * BLK:(kt + 1) * BLK],
                            xbf_h[rows, kt * 128:(kt + 1) * 128])
                    xtbs[bk] = xtb

                def lhsT(t, kt):
                    bk, j = divmod(t, BT)
                    base = kt * BLK + j * 128
                    return xtbs[bk][:, base:base + 128]

                def emit_mm(t, ci):
                    """32-matmul accumulation chunk for one token tile."""
                    coff, csz = CHUNKS[ci]
                    ps = psum_pool.tile([128, 512], F32, tag=f"pso{ci}",
                                        name=f"pso{ci}")
                    psos_map[(t, ci)] = ps
                    emitted.add((t, ci))
                    for kt in range(KT):
                        nc.tensor.matmul(
                            ps[:, :csz], lhsT(t, kt),
                            wT_r[:, kt, coff:coff + csz],
                            start=(kt == 0), stop=(kt == KT - 1))

                def emit_bias(t, ci):
                    if t not in osbs:
                        osbs[t] = osbp.tile([128, DSH], F32, tag="osb",
                                            name="osb")
                    coff, csz = CHUNKS[ci]
                    nc.vector.tensor_add(
                        osbs[t][:, coff:coff + csz],
                        psos_map.pop((t, ci))[:, :csz],
                        bias_rep[:, coff:coff + csz])

                def emit_store(t):
                    nc.gpsimd.dma_start(
                        out_h[t * 128:(t + 1) * 128, :], osbs.pop(t))

                emit_cast(0)
                if n_blocks > 1:
                    emit_cast(1)
                emit_xbar(0)

                # ---- weight prep in quarter-K stages; PE transposes via
                #      identity so staged matmuls see ordinary engine deps ----
                NQ = 4
                KQ = K // NQ
                for dt_i, (doff, p) in enumerate(dout_tiles):
                    for h in range(NQ):
                        ks = slice(h * KQ, (h + 1) * KQ)
                        wf = wp.tile([128, KQ], F32, tag="wf", name="wf")
                        mk = wp.tile([128, KQ], U8, tag="mk", name="mk")
                        sgn = wp.tile([128, KQ], BF16, tag="sgn", name="sgn")
                        wsb = wp.tile([128, KQ], BF16, tag="wsb", name="wsb")
                        nc.scalar.dma_start(wf[:p], w_h[doff:doff + p, ks])
                        nc.scalar.dma_start(mk[:p], mk_h[doff:doff + p, ks])
                        nc.scalar.sign(sgn[:p], wf[:p])
                        nc.scalar.mul(wsb[:p], sgn[:p], scale_vec[:p])
                        nc.vector.copy_predicated(wsb[:p], mk[:p], wf[:p])
                        for g4 in range(KQ // 512):
                            psw = psum_pool.tile([128, 512], BF16,
                                                 tag="psw", name="psw")
                            for j in range(4):
                                kl = g4 * 4 + j
                                nc.tensor.transpose(
                                    psw[:, j * 128:j * 128 + p],
                                    wsb[:p, kl * 128:(kl + 1) * 128],
                                    identity[:p, :p])
                            kt0 = h * (KQ // 128) + g4 * 4
                            nc.vector.tensor_copy(
                                wT_r[:, kt0:kt0 + 4, doff:doff + p],
                                psw.rearrange("a (j c) -> a j c",
                                              j=4)[:, :, :p])
                    if tok_tiles > n_staged:
                        # chunk 0 spans dout tiles 0-3, chunk 1 tiles 4-7
                        if dt_i == 3:
                            for t in range(n_staged):
                                emit_mm(t, 0)
                                emit_bias(t, 0)
                        elif dt_i == 7:
                            for t in range(n_staged):
                                emit_mm(t, 1)
                                emit_bias(t, 1)

                # bias broadcast deferred so it doesn't occupy the gpsimd
                # queue ahead of the first x cast-DMAs
                nc.gpsimd.dma_start(
                    out=bias_rep,
                    in_=b_h[:].rearrange("(a d) -> a d",
                                         a=1).to_broadcast((128, DSH)))

                # finish staged tiles: any chunk not yet emitted (covers both
                # chunk 2 and the tok_tiles <= n_staged case)
                for t in range(n_staged):
                    for ci in range(len(CHUNKS)):
                        if (t, ci) not in emitted:
                            emit_mm(t, ci)
                            emit_bias(t, ci)
                    emit_store(t)

                # ---- steady state over blocks; kt-major chunk interleave so
                # the three chunk matmuls of one (t, kt) share the PE
                # stationary and LDW dedupe drops 2 of 3 weight loads ----
                if n_blocks > 2:
                    emit_cast(2)
                if n_blocks > 1:
                    emit_xbar(1)
                for bk in range(1, n_blocks):
                    if bk + 2 < n_blocks:
                        emit_cast(bk + 2)
                    if bk + 1 < n_blocks:
                        emit_xbar(bk + 1)
                    for t in range(bk * BT, (bk + 1) * BT):
                        psos = {}
                        for ci, (coff, csz) in enumerate(CHUNKS):
                            psos[ci] = psum_pool.tile([128, 512], F32,
                                                      tag=f"pso{ci}",
                                                      name=f"pso{ci}")
                            psos_map[(t, ci)] = psos[ci]
                        for kt in range(KT):
                            for ci, (coff, csz) in enumerate(CHUNKS):
                                nc.tensor.matmul(
                                    psos[ci][:, :csz], lhsT(t, kt),
                                    wT_r[:, kt, coff:coff + csz],
                                    start=(kt == 0), stop=(kt == KT - 1))
                        for ci in range(len(CHUNKS)):
                            emit_bias(t, ci)
                        emit_store(t)
                    xtbs.pop(bk - 1, None)

    _dedupe_ldweights(nc)
    _split_excess_waits(nc)
    return nc


_NC_CACHE = {}


def _get_nc(m_tokens: int = M):
    if m_tokens not in _NC_CACHE:
        _NC_CACHE[m_tokens] = build_nc(m_tokens)
    return _NC_CACHE[m_tokens]


def _make_in_maps(x, weight, bias, outlier_mask, binary_scale):
    m_tokens = x.shape[0] * x.shape[1] if x.ndim == 3 else x.shape[0]
    xf = np.ascontiguousarray(x.reshape(m_tokens, K), dtype=np.float32)
    w = np.ascontiguousarray(weight, dtype=np.float32)
    b = np.ascontiguousarray(bias, dtype=np.float32)
    mk = np.ascontiguousarray(outlier_mask).view(np.uint8)
    sc = np.ascontiguousarray(binary_scale, dtype=np.float32).reshape(1, 1)
    ident = np.eye(128, dtype=ml_dtypes.bfloat16)
    in_maps = []
    for i in range(NCORES):
        sl = slice(i * DSH, (i + 1) * DSH)
        in_maps.append({
            "x": xf,
            "weight": np.ascontiguousarray(w[sl]),
            "bias": np.ascontiguousarray(b[sl]),
            "outlier_mask": np.ascontiguousarray(mk[sl]),
            "binary_scale": sc,
            "identity_const": ident,
        })
    return in_maps, m_tokens


def run_sharded(x, weight, bias, outlier_mask, binary_scale, trace=False):
    """Run on 8 cores; returns (full_output [M, DOUT] f32, BassKernelResults)."""
    in_maps, m_tokens = _make_in_maps(x, weight, bias, outlier_mask, binary_scale)
    nc = _get_nc(m_tokens)
    res = run_bass_kernel_spmd(nc, in_maps, core_ids=list(range(NCORES)),
                               trace=trace)
    full = np.concatenate([res.results[i]["out"] for i in range(NCORES)], axis=1)
    return full, res


def kernel(x, weight, bias, outlier_mask, binary_scale):
    full, _ = run_sharded(x, weight, bias, outlier_mask, binary_scale)
    return full.reshape(x.shape[0], x.shape[1], DOUT) if x.ndim == 3 else full
